# revision 1
# baseline (speedup 1.0000x reference)
"""Single-head causal attention on 8 TRN2 NeuronCores.

out[b,t,:] = softmax_causal((x Wq^T)(x Wk^T)^T / sqrt(C)) @ (x Wv^T)

Sharding: core = (batch b=core//2, parity p=core%2). Each core owns the
interleaved q-512-blocks g in {p, p+2, p+4, p+6} of its batch. One uniform
SPMD program: per q-position i the main (strictly-below-diagonal) phase runs
a fixed EMAIN[i] = [4,12,20,28] key-chunk extents; parity-0 cores get 4
zero-padded key chunks prepended host-side, with an indicator column (1=real
key, 0=pad) that feeds the softmax denominator, so pads contribute exactly
zero. The 4 diagonal chunks per q-block are handled by a separate static
phase with precomputed triangular masks.

Scores are produced transposed (S^T[s, tq] = k^T.T @ q^T) so that the
softmax denominator comes out of the PV matmul via an extra v-column
(the indicator), with no per-tile transposes of the probability matrix.
All matmul operands bf16; accumulation fp32; softmax has no max-subtraction
(scores are O(1) for this problem's distribution, exp is safe).
"""

import math
import os
import sys

for _p in ("/opt/trn_rl_repo",):
    if _p not in sys.path:
        sys.path.insert(0, _p)

import numpy as np
import ml_dtypes

BF16 = ml_dtypes.bfloat16

B, T, C, H = 4, 4096, 1024, 64
NCORES = 8
SCALE = C ** -0.5

QB = 512                    # q block width (columns of q^T per block)
NQB = 4                     # q blocks per core (4 * 512 = 2048 rows)
EMAIN = (4, 12, 20, 28)     # uniform main-phase extents (128-key chunks)
MAINC = 28                  # main kv chunks per core (28*128 = 3584 cols)
DIAGC = 16                  # diag kv chunks per core (owns its 2048 q rows)
VN = MAINC + DIAGC          # 44 chunks in v-natural storage

_CACHE = {}


def _build_program():
    import concourse.bass as bass
    import concourse.mybir as mybir
    import concourse.tile as tile
    from concourse import bacc
    from concourse.masks import make_identity

    f32 = mybir.dt.float32
    bf16 = mybir.dt.bfloat16

    nc = bacc.Bacc("TRN2", target_bir_lowering=False, debug=False)
    xq_d = nc.dram_tensor("xq", [C, NQB * QB], bf16, kind="ExternalInput")
    xkv_d = nc.dram_tensor("xkv", [C, MAINC * 128], bf16, kind="ExternalInput")
    wt_d = nc.dram_tensor("wt", [C, 192], bf16, kind="ExternalInput")
    ind_d = nc.dram_tensor("ind", [512, 1], f32, kind="ExternalInput")
    out_d = nc.dram_tensor("out", [NQB * QB, H], f32, kind="ExternalOutput")

    NQ = NQB * QB            # 2048
    NM = MAINC * 128         # 3584

    with tile.TileContext(nc) as tc:
        with tc.tile_pool(name="persist", bufs=1) as P:
            xq_sb = P.tile([128, 8 * NQ], bf16)
            xkv_sb = P.tile([128, 8 * NM], bf16)
            w_sb = P.tile([128, 8 * 192], bf16)
            qT_sb = P.tile([64, NQ], bf16)
            kq_sb = P.tile([64, NQ], bf16)       # diag keys k^T
            km_sb = P.tile([64, NM], bf16)       # main keys k^T
            vTq_sb = P.tile([64, NQ], bf16)
            vTm_sb = P.tile([64, NM], bf16)
            vn_sb = P.tile([128, VN * 65], bf16)  # v natural + indicator col
            mask_sb = P.tile([128, 896], bf16)    # sliding triangular master
            idb_sb = P.tile([128, 128], bf16)     # bf16 identity
            idf_sb = P.tile([128, 128], f32)      # f32 identity
            ind_sb = P.tile([128, 4], f32)        # pad indicator, chunks 0-3

            # --- constants -------------------------------------------------
            make_identity(nc, idb_sb[:, :])
            make_identity(nc, idf_sb[:, :])
            # master mask: valid (keep 1.0) iff ycol >= x + 384
            nc.gpsimd.memset(mask_sb[:, :], 1.0)
            nc.gpsimd.affine_select(
                out=mask_sb[:, :],
                in_=mask_sb[:, :],
                compare_op=mybir.AluOpType.is_ge,
                fill=0.0,
                base=-384,
                pattern=[[1, 896]],
                channel_multiplier=-1,
            )
            nc.gpsimd.memset(vn_sb[:, :], 1.0)

            # --- input DMAs (one instruction each: chunk-major rearrange) --
            nc.sync.dma_start(
                out=xq_sb.rearrange("p (c n) -> p c n", c=8),
                in_=xq_d.rearrange("(c p) n -> p c n", p=128))
            nc.sync.dma_start(
                out=xkv_sb.rearrange("p (c n) -> p c n", c=8),
                in_=xkv_d.rearrange("(c p) n -> p c n", p=128))
            nc.sync.dma_start(
                out=w_sb.rearrange("p (c n) -> p c n", c=8),
                in_=wt_d.rearrange("(c p) n -> p c n", p=128))
            nc.sync.dma_start(
                out=ind_sb.rearrange("p (j o) -> p j o", j=4),
                in_=ind_d.rearrange("(j p) o -> p j o", p=128))

            # --- phase 1: projections -------------------------------------
            # q^T/k^T/v^T = W^T.T @ x^T per 512-col block, contracting C.
            with tc.tile_pool(name="psum1", bufs=2, space="PSUM") as PS1:
                for src_sb, ncols, has_q in (
                    (xq_sb, NQ, True),
                    (xkv_sb, NM, False),
                ):
                    for tb in range(ncols // 512):
                        pt = PS1.tile([64, 1536], f32, tag="projps")
                        for c in range(8):
                            rhs = src_sb[:, c * ncols + tb * 512:
                                         c * ncols + tb * 512 + 512]
                            st = c == 0
                            sp = c == 7
                            if has_q:
                                nc.tensor.matmul(
                                    pt[:, 0:512],
                                    w_sb[:, c * 192:c * 192 + 64],
                                    rhs, start=st, stop=sp)
                            nc.tensor.matmul(
                                pt[:, 512:1024],
                                w_sb[:, c * 192 + 64:c * 192 + 128],
                                rhs, start=st, stop=sp)
                            nc.tensor.matmul(
                                pt[:, 1024:1536],
                                w_sb[:, c * 192 + 128:c * 192 + 192],
                                rhs, start=st, stop=sp)
                        sl = slice(tb * 512, tb * 512 + 512)
                        if has_q:
                            nc.vector.tensor_copy(qT_sb[:, sl], pt[:, 0:512])
                            nc.vector.tensor_copy(kq_sb[:, sl], pt[:, 512:1024])
                            nc.vector.tensor_copy(vTq_sb[:, sl], pt[:, 1024:1536])
                        else:
                            nc.vector.tensor_copy(km_sb[:, sl], pt[:, 512:1024])
                            nc.vector.tensor_copy(vTm_sb[:, sl], pt[:, 1024:1536])

            # --- phase 2: v^T -> v natural (+ indicator col) ---------------
            with tc.tile_pool(name="psum2", bufs=3, space="PSUM") as PS2:
                for j in range(VN):
                    if j < MAINC:
                        vsrc = vTm_sb[:, j * 128:(j + 1) * 128]
                    else:
                        jj = j - MAINC
                        vsrc = vTq_sb[:, jj * 128:(jj + 1) * 128]
                    tp = PS2.tile([128, 64], bf16, tag="vtr")
                    nc.tensor.transpose(tp, vsrc, idb_sb[0:64, 0:64])
                    nc.vector.tensor_copy(vn_sb[:, j * 65:j * 65 + 64], tp)

            # --- phase 3: attention ---------------------------------------
            with tc.tile_pool(name="psc", bufs=4, space="PSUM") as PSC, \
                 tc.tile_pool(name="pout", bufs=2, space="PSUM") as POUT, \
                 tc.tile_pool(name="ptr", bufs=2, space="PSUM") as PTR, \
                 tc.tile_pool(name="work", bufs=4) as W, \
                 tc.tile_pool(name="fin", bufs=4) as F:
                for qb in range(NQB):
                    op = POUT.tile([65, 512], f32, tag="op")
                    qAP = qT_sb[:, qb * QB:(qb + 1) * QB]
                    nmain = EMAIN[qb]
                    nslots = nmain + 4
                    for t in range(nslots):
                        if t < nmain:
                            kAP = km_sb[:, t * 128:(t + 1) * 128]
                            vcol = t
                        else:
                            d = t - nmain
                            j = qb * 4 + d
                            kAP = kq_sb[:, j * 128:(j + 1) * 128]
                            vcol = MAINC + j
                        sp = PSC.tile([128, 512], f32, tag="sc")
                        nc.tensor.matmul(sp, kAP, qAP, start=True, stop=True)
                        pb = W.tile([128, 512], bf16, tag="pb")
                        nc.scalar.activation(
                            pb, sp, mybir.ActivationFunctionType.Exp,
                            scale=SCALE)
                        if t >= nmain:
                            d = t - nmain
                            nc.vector.tensor_mul(
                                pb, pb,
                                mask_sb[:, 384 - d * 128:896 - d * 128])
                        elif t < 4:
                            # pads only ever live in physical chunks 0-3
                            nc.vector.tensor_scalar_mul(
                                pb, pb, ind_sb[:, t:t + 1])
                        nc.tensor.matmul(
                            op, vn_sb[:, vcol * 65:(vcol + 1) * 65], pb,
                            start=(t == 0), stop=(t == nslots - 1))
                    # finalize: transpose [65,512] -> 4x [128,65], divide, DMA
                    ob = F.tile([65, 512], f32, tag="ob")
                    nc.vector.tensor_copy(ob, op)
                    for t4 in range(4):
                        tp = PTR.tile([128, 65], f32, tag="otr")
                        nc.tensor.transpose(
                            tp, ob[:, t4 * 128:(t4 + 1) * 128],
                            idf_sb[0:65, 0:65])
                        rc = F.tile([128, 1], f32, tag="rc")
                        nc.vector.reciprocal(rc, tp[:, 64:65])
                        rs = F.tile([128, 64], f32, tag="rs")
                        nc.vector.tensor_scalar_mul(rs, tp[:, 0:64], rc)
                        r0 = qb * QB + t4 * 128
                        nc.sync.dma_start(
                            out=out_d[r0:r0 + 128, :], in_=rs)
    nc.compile()
    return nc


def _get_program():
    if "nc" not in _CACHE:
        _CACHE["nc"] = _build_program()
    return _CACHE["nc"]


def _host_prep(x, Wk, Wq, Wv):
    """Build the 8 per-core input maps."""
    wt = np.concatenate([Wq.T, Wk.T, Wv.T], axis=1).astype(BF16)  # [C, 192]
    xT = [np.ascontiguousarray(x[b].T).astype(BF16) for b in range(B)]
    in_maps = []
    for core in range(NCORES):
        b, p = core // 2, core % 2
        gs = [2 * i + p for i in range(NQB)]
        xq = np.concatenate(
            [xT[b][:, 512 * g:512 * g + 512] for g in gs], axis=1)
        if p == 0:
            xkv = np.concatenate(
                [np.zeros((C, 512), dtype=BF16), xT[b][:, 0:3072]], axis=1)
            ind = np.zeros(512, dtype=np.float32)
        else:
            xkv = np.ascontiguousarray(xT[b][:, 0:3584])
            ind = np.ones(512, dtype=np.float32)
        in_maps.append({
            "xq": np.ascontiguousarray(xq),
            "xkv": np.ascontiguousarray(xkv),
            "wt": np.ascontiguousarray(wt),
            "ind": ind.reshape(-1, 1),
        })
    return in_maps


def _gather(results):
    out = np.zeros((B, T, H), dtype=np.float32)
    for core in range(NCORES):
        b, p = core // 2, core % 2
        shard = np.asarray(results[core]["out"], dtype=np.float32)
        for i in range(NQB):
            g = 2 * i + p
            out[b, 512 * g:512 * g + 512, :] = shard[512 * i:512 * i + 512, :]
    return out


def run(x, Wk, Wq, Wv, trace=False):
    from concourse.bass_utils import run_bass_kernel_spmd

    nc = _get_program()
    in_maps = _host_prep(x, Wk, Wq, Wv)
    res = run_bass_kernel_spmd(
        nc, in_maps, list(range(NCORES)), trace=trace)
    return _gather(res.results), res


def kernel(x, Wk, Wq, Wv):
    out, _ = run(np.asarray(x, dtype=np.float32),
                 np.asarray(Wk, dtype=np.float32),
                 np.asarray(Wq, dtype=np.float32),
                 np.asarray(Wv, dtype=np.float32))
    return out



# revision 13
# speedup vs baseline: 1.0689x; 1.0689x over previous
"""Single-head causal attention on 8 TRN2 NeuronCores.

out[b,t,:] = softmax_causal((x Wq^T)(x Wk^T)^T / sqrt(C)) @ (x Wv^T)

Sharding: core = (batch b=core//2, parity p=core%2). Each core owns the
interleaved q-512-blocks g in {p, p+2, p+4, p+6} of its batch. One uniform
SPMD program: per q-slot i the main (strictly-below-diagonal) phase runs a
fixed EMAIN[i] = [4,12,20,28] key-chunk extents; parity-0 cores get 4
zero-padded key chunks prepended host-side, killed by a per-partition -BIG
bias fused into the exp activation (pads -> exp -> 0), so they contribute
exactly zero to both numerator and denominator (65th v column of ones).

Pipeline structure (single in-order PE stream, ~everything overlapped):
 - inputs DMA'd in 13 per-512-block chunks ordered by first use, so the
   first projection starts ~4us in and all later loads hide under compute.
 - projections M-packed: (q|k) and (k|v) as single M=128 matmuls.
 - v^T -> v natural via XBAR dma-transpose (SBUF->SBUF, off the PE).
 - attention emitted software-pipelined: score(i+2) runs ahead of PV(i),
   with projection matmuls for later q-blocks interleaved as fillers into
   the ACT(exp)-bound stretches; diagonal tiles are column-shrunk to skip
   fully-masked queries, triangular window masked on the Pool engine.
Scores produced transposed (S^T[s,tq]) so the softmax denominator falls
out of the PV matmul's extra indicator column; no probability transposes.
All matmul operands bf16; accumulation fp32; no max-subtraction (scores
are O(1) for this distribution; exp is safe).
"""

import math
import os
import sys

for _p in ("/opt/trn_rl_repo",):
    if _p not in sys.path:
        sys.path.insert(0, _p)

import numpy as np
import ml_dtypes

BF16 = ml_dtypes.bfloat16

B, T, C, H = 4, 4096, 1024, 64
NCORES = 8
SCALE = C ** -0.5
NEGBIG = -30000.0

QB = 512                    # q block width (columns of q^T per block)
NQB = 4                     # q blocks per core (4 * 512 = 2048 rows)
EMAIN = (4, 12, 20, 28)     # uniform main-phase extents (128-key chunks)
MAINC = 28                  # main kv chunks per core (28*128 = 3584 cols)
DIAGC = 16                  # diag kv chunks per core (owns its 2048 q rows)
VN = MAINC + DIAGC          # 44 chunks in v-natural storage
NQ = NQB * QB               # 2048
NM = MAINC * 128            # 3584

_CACHE = {}


def _build_program():
    import concourse.bass as bass
    import concourse.mybir as mybir
    import concourse.tile as tile
    from concourse import bacc
    from concourse.masks import make_identity

    f32 = mybir.dt.float32
    bf16 = mybir.dt.bfloat16

    nc = bacc.Bacc("TRN2", target_bir_lowering=False, debug=False)
    xq_d = nc.dram_tensor("xq", [C, NQ], bf16, kind="ExternalInput")
    xkv_d = nc.dram_tensor("xkv", [C, NM], bf16, kind="ExternalInput")
    wt_d = nc.dram_tensor("wt", [C, 192], bf16, kind="ExternalInput")
    bias_d = nc.dram_tensor("bias", [128, 4], f32, kind="ExternalInput")
    out_d = nc.dram_tensor("out", [NQ, H], f32, kind="ExternalOutput")

    with tile.TileContext(nc) as tc:
        with tc.tile_pool(name="persist", bufs=1) as P, \
             tc.tile_pool(name="psum", bufs=1, space="PSUM") as PS, \
             tc.tile_pool(name="work", bufs=1) as W:
            # ---- persistent SBUF -----------------------------------------
            xq_sb = P.tile([128, 8 * NQ], bf16)
            xkv_sb = P.tile([128, 8 * NM], bf16)
            w_sb = P.tile([128, 8 * 192], bf16)
            qT_sb = P.tile([64, NQ], bf16)        # q^T
            kq_sb = P.tile([64, NQ], bf16)        # diag k^T
            kv_sb = P.tile([128, NM], bf16)       # rows 0-63 k^T,  64-127 v^T
            vq_sb = P.tile([64, NQ], bf16)        # diag v^T staging
            vn_sb = P.tile([128, VN * 80], bf16)  # v nat (64) + ones col @64, stride 80
            tri_sb = P.tile([128, 128], bf16)     # causal window (keep t>=s)
            idf_sb = P.tile([128, 128], f32)      # f32 identity (out transposes)
            bias_sb = P.tile([128, 4], f32)       # exp bias (-BIG on pad chunks)

            # ---- constants -----------------------------------------------
            make_identity(nc, idf_sb[:, :])
            nc.gpsimd.memset(vn_sb[:, :], 1.0)
            nc.gpsimd.memset(tri_sb[:, :], 1.0)
            nc.gpsimd.affine_select(
                out=tri_sb[:, :], in_=tri_sb[:, :],
                compare_op=mybir.AluOpType.is_ge, fill=0.0,
                base=0, pattern=[[1, 128]], channel_multiplier=-1)

            # ---- input DMAs: per-512-block chunks, ordered by first use --
            xq_r3 = xq_sb.rearrange("p (c n) -> p c n", c=8)
            xq_d3 = xq_d.rearrange("(c p) n -> p c n", p=128)
            xkv_r3 = xkv_sb.rearrange("p (c n) -> p c n", c=8)
            xkv_d3 = xkv_d.rearrange("(c p) n -> p c n", p=128)

            def dma_xq(g):
                sl = slice(g * QB, (g + 1) * QB)
                nc.sync.dma_start(out=xq_r3[:, :, sl], in_=xq_d3[:, :, sl])

            def dma_kv(b):
                sl = slice(b * 512, (b + 1) * 512)
                nc.sync.dma_start(out=xkv_r3[:, :, sl], in_=xkv_d3[:, :, sl])

            nc.sync.dma_start(
                out=w_sb.rearrange("p (c n) -> p c n", c=8),
                in_=wt_d.rearrange("(c p) n -> p c n", p=128))
            dma_kv(0)
            dma_xq(0)
            nc.sync.dma_start(out=bias_sb, in_=bias_d[:, :])
            dma_xq(1)
            dma_kv(1)
            dma_kv(2)
            dma_xq(2)
            dma_kv(3)
            dma_kv(4)
            dma_xq(3)
            dma_kv(5)
            dma_kv(6)

            # ---- projection emitters (filler items: (weight, closure)) ---
            # w_sb layout per chunk c: [q 0:64 | k 64:128 | v 128:192]
            def proj_xq_items(g):
                items = []
                pt = {}

                def mk_qk(c):
                    def f():
                        if c == 0:
                            pt["qk"] = PS.tile([128, 512], f32, tag="pt",
                                               bufs=2, name="ptqk")
                        nc.tensor.matmul(
                            pt["qk"],
                            w_sb[:, c * 192:c * 192 + 128],
                            xq_sb[:, c * NQ + g * QB:c * NQ + (g + 1) * QB],
                            start=(c == 0), stop=(c == 7))
                    return f

                def mk_v(c):
                    def f():
                        if c == 0:
                            pt["v"] = PS.tile([64, 512], f32, tag="ptv",
                                              bufs=1, name="ptv")
                        nc.tensor.matmul(
                            pt["v"],
                            w_sb[:, c * 192 + 128:c * 192 + 192],
                            xq_sb[:, c * NQ + g * QB:c * NQ + (g + 1) * QB],
                            start=(c == 0), stop=(c == 7))
                    return f

                def cp_qk():
                    nc.vector.tensor_copy(
                        qT_sb[0:64, g * QB:(g + 1) * QB], pt["qk"][0:64, :])
                    nc.vector.tensor_copy(
                        kq_sb[0:64, g * QB:(g + 1) * QB], pt["qk"][64:128, :])

                def cp_v():
                    nc.vector.tensor_copy(
                        vq_sb[0:64, g * QB:(g + 1) * QB], pt["v"])

                def mk_tr(d):
                    def f():
                        j = MAINC + 4 * g + d
                        nc.sync.dma_start_transpose(
                            out=vn_sb[:, j * 80:j * 80 + 64],
                            in_=vq_sb[0:64, (4 * g + d) * 128:
                                      (4 * g + d + 1) * 128])
                    return f

                for c in range(8):
                    items.append((1, mk_qk(c)))
                items.append((0, cp_qk))
                for c in range(8):
                    items.append((1, mk_v(c)))
                items.append((0, cp_v))
                for d in range(4):
                    items.append((0, mk_tr(d)))
                return items

            def proj_kv_items(b):
                items = []
                pt = {}

                def mk(c):
                    def f():
                        if c == 0:
                            pt["kv"] = PS.tile([128, 512], f32, tag="pt",
                                               bufs=2, name="ptkv")
                        nc.tensor.matmul(
                            pt["kv"],
                            w_sb[:, c * 192 + 64:c * 192 + 192],
                            xkv_sb[:, c * NM + b * 512:c * NM + (b + 1) * 512],
                            start=(c == 0), stop=(c == 7))
                    return f

                def cp():
                    nc.vector.tensor_copy(
                        kv_sb[:, b * 512:(b + 1) * 512], pt["kv"])

                def mk_tr(cc):
                    def f():
                        j = 4 * b + cc
                        nc.sync.dma_start_transpose(
                            out=vn_sb[:, j * 80:j * 80 + 64],
                            in_=kv_sb[64:128, j * 128:(j + 1) * 128])
                    return f

                for c in range(8):
                    items.append((1, mk(c)))
                items.append((0, cp))
                for cc in range(4):
                    items.append((0, mk_tr(cc)))
                return items

            # ---- finalize emitter ----------------------------------------
            def fin_items(g, op):
                items = []
                st = {}

                def cp_ob():
                    st["ob"] = W.tile([65, 512], f32, tag="ob", bufs=2,
                                      name="ob")
                    nc.vector.tensor_copy(st["ob"], op[0:65, :])

                def mk_t4(t4):
                    def f():
                        tp = PS.tile([128, 65], f32, tag="fin", bufs=1,
                                     name="otr")
                        nc.tensor.transpose(
                            tp, st["ob"][:, t4 * 128:(t4 + 1) * 128],
                            idf_sb[0:65, 0:65])
                        rc = W.tile([128, 1], f32, tag="rc", bufs=2, name="rc")
                        nc.vector.reciprocal(rc, tp[:, 64:65])
                        rs = W.tile([128, 64], f32, tag="rs", bufs=2, name="rs")
                        nc.vector.tensor_scalar_mul(rs, tp[:, 0:64], rc)
                        r0 = g * QB + t4 * 128
                        nc.sync.dma_start(out=out_d[r0:r0 + 128, :], in_=rs)
                    return f

                items.append((0, cp_ob))
                for t4 in range(4):
                    items.append((1, mk_t4(t4)))
                return items

            # ---- attention -----------------------------------------------
            # Slots per qb g: EMAIN[g] main chunks then 4 diag chunks.
            # Diag d covers local keys [128d,128d+128); only queries
            # t >= 128d see it, so score/exp/PV are shrunk to W=512-128d.
            def emit_score_exp(g, slot):
                kind, t = slot
                sc = PS.tile([128, 512], f32, tag="sc", bufs=2, name="sc")
                pb = W.tile([128, 512], bf16, tag="pb", bufs=4, name="pb")
                if kind == "main":
                    nc.tensor.matmul(
                        sc, kv_sb[0:64, t * 128:(t + 1) * 128],
                        qT_sb[0:64, g * QB:(g + 1) * QB],
                        start=True, stop=True)
                    bias = bias_sb[:, t:t + 1] if t < 4 else 0.0
                    nc.scalar.activation(
                        pb, sc, mybir.ActivationFunctionType.Exp,
                        scale=SCALE, bias=bias)
                    return pb, 512
                d = t
                w = 512 - 128 * d
                j = 4 * g + d
                nc.tensor.matmul(
                    sc[:, 0:w], kq_sb[0:64, j * 128:(j + 1) * 128],
                    qT_sb[0:64, g * QB + 128 * d:(g + 1) * QB],
                    start=True, stop=True)
                nc.scalar.activation(
                    pb[:, 0:w], sc[:, 0:w],
                    mybir.ActivationFunctionType.Exp, scale=SCALE)
                nc.gpsimd.tensor_mul(pb[:, 0:128], pb[:, 0:128], tri_sb)
                return pb, w

            def emit_pv(g, slot, op, pb, w, nslots, i):
                kind, t = slot
                if kind == "main":
                    nc.tensor.matmul(
                        op[0:65, :], vn_sb[:, t * 80:t * 80 + 65], pb,
                        start=(i == 0), stop=False, skip_group_check=True)
                else:
                    d = t
                    j = MAINC + 4 * g + d
                    nc.tensor.matmul(
                        op[0:65, 128 * d:512], vn_sb[:, j * 80:j * 80 + 65],
                        pb[:, 0:w], start=False, stop=(i == nslots - 1),
                        skip_group_check=True)

            # ---- top-level schedule --------------------------------------
            # upfront projections
            for _, f in proj_kv_items(0):
                f()
            for _, f in proj_xq_items(0):
                f()

            fillers = []        # (weight, closure) queue, consumed in order

            def add_fillers(items):
                fillers.extend(items)

            def pump(target):
                # consume fillers until cumulative weight >= target
                while fillers and pump.done < target:
                    wgt, f = fillers.pop(0)
                    f()
                    pump.done += wgt
                while fillers and fillers[0][0] == 0:
                    _, f = fillers.pop(0)
                    f()
            pump.done = 0

            qb_fill = {
                0: lambda: proj_xq_items(1) + proj_kv_items(1)
                + proj_kv_items(2),
                1: lambda: proj_xq_items(2) + proj_kv_items(3)
                + proj_kv_items(4),
                2: lambda: proj_xq_items(3),
                3: lambda: proj_kv_items(5) + proj_kv_items(6),
            }
            # for qb3 the kv5/kv6 fillers must land before slots 20/24: pace
            # them over the first ~16 tiles via an early virtual deadline.
            pace_tiles = {0: 8, 1: 16, 2: 24, 3: 16}

            for g in range(NQB):
                op = PS.tile([128, 512], f32, tag="op", bufs=2, name="op")
                slots = [("main", t) for t in range(EMAIN[g])] \
                    + [("diag", d) for d in range(4)]
                n = len(slots)
                base = pump.done
                add_fillers(qb_fill[g]())
                wsum = sum(w for w, _ in fillers)
                pend = []   # (slot, pb, w, i) awaiting PV
                npace = pace_tiles[g]
                for i, slot in enumerate(slots):
                    pump(base + math.ceil(wsum * min(1.0, (i + 1) / npace)))
                    if len(pend) >= 2:
                        s2, pb2, w2, i2 = pend.pop(0)
                        emit_pv(g, s2, op, pb2, w2, n, i2)
                    pb, w = emit_score_exp(g, slot)
                    pend.append((slot, pb, w, i))
                for s2, pb2, w2, i2 in pend:
                    emit_pv(g, s2, op, pb2, w2, n, i2)
                add_fillers(fin_items(g, op))

            while fillers:
                _, f = fillers.pop(0)
                f()
    nc.compile()
    return nc


def _get_program():
    if "nc" not in _CACHE:
        _CACHE["nc"] = _build_program()
    return _CACHE["nc"]


def _host_prep(x, Wk, Wq, Wv):
    """Build the 8 per-core input maps."""
    wt = np.concatenate([Wq.T, Wk.T, Wv.T], axis=1).astype(BF16)  # [C, 192]
    xT = [np.ascontiguousarray(x[b].T).astype(BF16) for b in range(B)]
    bias_pad = np.full((128, 4), NEGBIG, dtype=np.float32)
    bias_real = np.zeros((128, 4), dtype=np.float32)
    in_maps = []
    for core in range(NCORES):
        b, p = core // 2, core % 2
        gs = [2 * i + p for i in range(NQB)]
        xq = np.concatenate(
            [xT[b][:, 512 * g:512 * g + 512] for g in gs], axis=1)
        if p == 0:
            xkv = np.concatenate(
                [np.zeros((C, 512), dtype=BF16), xT[b][:, 0:3072]], axis=1)
            bias = bias_pad
        else:
            xkv = np.ascontiguousarray(xT[b][:, 0:3584])
            bias = bias_real
        in_maps.append({
            "xq": np.ascontiguousarray(xq),
            "xkv": np.ascontiguousarray(xkv),
            "wt": np.ascontiguousarray(wt),
            "bias": bias,
        })
    return in_maps


def _gather(results):
    out = np.zeros((B, T, H), dtype=np.float32)
    for core in range(NCORES):
        b, p = core // 2, core % 2
        shard = np.asarray(results[core]["out"], dtype=np.float32)
        for i in range(NQB):
            g = 2 * i + p
            out[b, 512 * g:512 * g + 512, :] = shard[512 * i:512 * i + 512, :]
    return out


def run(x, Wk, Wq, Wv, trace=False):
    from concourse.bass_utils import run_bass_kernel_spmd

    nc = _get_program()
    in_maps = _host_prep(x, Wk, Wq, Wv)
    res = run_bass_kernel_spmd(
        nc, in_maps, list(range(NCORES)), trace=trace)
    return _gather(res.results), res


def kernel(x, Wk, Wq, Wv):
    out, _ = run(np.asarray(x, dtype=np.float32),
                 np.asarray(Wk, dtype=np.float32),
                 np.asarray(Wq, dtype=np.float32),
                 np.asarray(Wv, dtype=np.float32))
    return out


# revision 20
# speedup vs baseline: 1.7884x; 1.6732x over previous
"""Single-head causal attention on 8 TRN2 NeuronCores.

out[b,t,:] = softmax_causal((x Wq^T)(x Wk^T)^T / sqrt(C)) @ (x Wv^T)

Sharding: core = (batch b=core//2, parity p=core%2). Each core owns the
interleaved q-512-blocks g in {p, p+2, p+4, p+6} of its batch. One uniform
SPMD program: per q-slot i the main (strictly-below-diagonal) phase runs a
fixed EMAIN[i] = [4,12,20,28] key-chunk extents; parity-0 cores get 4
zero-padded key chunks prepended host-side, killed by a per-partition -BIG
bias fused into the exp activation (pads -> exp -> 0), so they contribute
exactly zero to both numerator and denominator (65th v column of ones).

Pipeline structure (single in-order PE stream, ~everything overlapped):
 - inputs DMA'd in 13 per-512-block chunks ordered by first use, so the
   first projection starts ~4us in and all later loads hide under compute.
 - projections M-packed: (q|k) and (k|v) as single M=128 matmuls.
 - v^T -> v natural via XBAR dma-transpose (SBUF->SBUF, off the PE).
 - attention emitted software-pipelined: score(i+2) runs ahead of PV(i),
   with projection matmuls for later q-blocks interleaved as fillers into
   the ACT(exp)-bound stretches; diagonal tiles are column-shrunk to skip
   fully-masked queries, triangular window masked on the Pool engine.
Scores produced transposed (S^T[s,tq]) so the softmax denominator falls
out of the PV matmul's extra indicator column; no probability transposes.
All matmul operands bf16; accumulation fp32; no max-subtraction (scores
are O(1) for this distribution; exp is safe).
"""

import math
import os
import sys

for _p in ("/opt/trn_rl_repo",):
    if _p not in sys.path:
        sys.path.insert(0, _p)

import numpy as np
import ml_dtypes

BF16 = ml_dtypes.bfloat16

B, T, C, H = 4, 4096, 1024, 64
NCORES = 8
SCALE = C ** -0.5
NEGBIG = -30000.0

QB = 512                    # q block width (columns of q^T per block)
NQB = 4                     # q blocks per core (4 * 512 = 2048 rows)
EMAIN = (4, 12, 20, 28)     # uniform main-phase extents (128-key chunks)
MAINC = 28                  # main kv chunks per core (28*128 = 3584 cols)
DIAGC = 16                  # diag kv chunks per core (owns its 2048 q rows)
VN = MAINC + DIAGC          # 44 chunks in v-natural storage
NQ = NQB * QB               # 2048
NM = MAINC * 128            # 3584

_CACHE = {}


def _build_program():
    import concourse.bass as bass
    import concourse.mybir as mybir
    import concourse.tile as tile
    from concourse import bacc
    from concourse.masks import make_identity

    f32 = mybir.dt.float32
    bf16 = mybir.dt.bfloat16

    nc = bacc.Bacc("TRN2", target_bir_lowering=False, debug=False)
    xq_d = nc.dram_tensor("xq", [C, NQ], bf16, kind="ExternalInput")
    xkv_d = nc.dram_tensor("xkv", [C, NM], bf16, kind="ExternalInput")
    wt_d = nc.dram_tensor("wt", [C, 192], bf16, kind="ExternalInput")
    bias_d = nc.dram_tensor("bias", [128, 4], f32, kind="ExternalInput")
    out_d = nc.dram_tensor("out", [NQ, H], f32, kind="ExternalOutput")

    with tile.TileContext(nc) as tc:
        with tc.tile_pool(name="persist", bufs=1) as P, \
             tc.tile_pool(name="psum", bufs=1, space="PSUM") as PS, \
             tc.tile_pool(name="work", bufs=1) as W:
            # ---- persistent SBUF -----------------------------------------
            xq_sb = P.tile([128, 8 * NQ], bf16)
            xkv_sb = P.tile([128, 8 * NM], bf16)
            w_sb = P.tile([128, 8 * 192], bf16)
            qT_sb = P.tile([64, NQ], bf16)        # q^T
            kq_sb = P.tile([64, NQ], bf16)        # diag k^T
            kv_sb = P.tile([128, NM], bf16)       # rows 0-63 k^T,  64-127 v^T
            vq_sb = P.tile([64, NQ], bf16)        # diag v^T staging
            vn_sb = P.tile([128, VN * 80], bf16)  # v nat (64) + ones col @64, stride 80
            tri_sb = P.tile([128, 128], bf16)     # causal window (keep t>=s)
            idf_sb = P.tile([128, 128], f32)      # f32 identity (out transposes)
            idb_sb = P.tile([128, 128], bf16)     # bf16 identity (v transposes)
            bias_sb = P.tile([128, 4], f32)       # exp bias (-BIG on pad chunks)

            # ---- constants -----------------------------------------------
            make_identity(nc, idf_sb[:, :])
            make_identity(nc, idb_sb[:, :])
            nc.gpsimd.memset(vn_sb[:, :], 1.0)
            nc.gpsimd.memset(tri_sb[:, :], 1.0)
            nc.gpsimd.affine_select(
                out=tri_sb[:, :], in_=tri_sb[:, :],
                compare_op=mybir.AluOpType.is_ge, fill=0.0,
                base=0, pattern=[[1, 128]], channel_multiplier=-1)

            # ---- input DMAs: per-512-block chunks, ordered by first use --
            xq_r3 = xq_sb.rearrange("p (c n) -> p c n", c=8)
            xq_d3 = xq_d.rearrange("(c p) n -> p c n", p=128)
            xkv_r3 = xkv_sb.rearrange("p (c n) -> p c n", c=8)
            xkv_d3 = xkv_d.rearrange("(c p) n -> p c n", p=128)

            def dma_xq(g):
                sl = slice(g * QB, (g + 1) * QB)
                nc.sync.dma_start(out=xq_r3[:, :, sl], in_=xq_d3[:, :, sl])

            def dma_kv(b):
                sl = slice(b * 512, (b + 1) * 512)
                nc.sync.dma_start(out=xkv_r3[:, :, sl], in_=xkv_d3[:, :, sl])

            nc.sync.dma_start(
                out=w_sb.rearrange("p (c n) -> p c n", c=8),
                in_=wt_d.rearrange("(c p) n -> p c n", p=128))
            dma_kv(0)
            dma_xq(0)
            nc.sync.dma_start(out=bias_sb, in_=bias_d[:, :])
            dma_xq(1)
            dma_kv(1)
            dma_kv(2)
            dma_xq(2)
            dma_kv(3)
            dma_kv(4)
            dma_xq(3)
            dma_kv(5)
            dma_kv(6)

            # ---- projection emitters (filler items: (weight, closure)) ---
            # w_sb layout per chunk c: [q 0:64 | k 64:128 | v 128:192]
            def proj_xq_items(g):
                items = []
                pt = {}

                def mk_qk(c):
                    def f():
                        if c == 0:
                            pt["qk"] = PS.tile([128, 512], f32, tag="pt",
                                               bufs=2, name="ptqk")
                        nc.tensor.matmul(
                            pt["qk"],
                            w_sb[:, c * 192:c * 192 + 128],
                            xq_sb[:, c * NQ + g * QB:c * NQ + (g + 1) * QB],
                            start=(c == 0), stop=(c == 7))
                    return f

                def mk_v(c):
                    def f():
                        if c == 0:
                            pt["v"] = PS.tile([128, 512], f32, tag="pt",
                                              bufs=2, name="ptv")
                        nc.tensor.matmul(
                            pt["v"][0:64, :],
                            w_sb[:, c * 192 + 128:c * 192 + 192],
                            xq_sb[:, c * NQ + g * QB:c * NQ + (g + 1) * QB],
                            start=(c == 0), stop=(c == 7))
                    return f

                def cp_qk():
                    nc.vector.tensor_copy(
                        qT_sb[0:64, g * QB:(g + 1) * QB], pt["qk"][0:64, :])
                    nc.vector.tensor_copy(
                        kq_sb[0:64, g * QB:(g + 1) * QB], pt["qk"][64:128, :])

                def cp_v():
                    nc.vector.tensor_copy(
                        vq_sb[0:64, g * QB:(g + 1) * QB], pt["v"][0:64, :])

                for c in range(8):
                    items.append((1, mk_qk(c)))
                items.append((0, cp_qk))
                for c in range(8):
                    items.append((1, mk_v(c)))
                items.append((0, cp_v))
                for d in range(4):
                    items.append((1, mk_vtr(
                        MAINC + 4 * g + d,
                        vq_sb[0:64, (4 * g + d) * 128:(4 * g + d + 1) * 128],
                        False)))
                return items

            def proj_kv_items(b):
                items = []
                pt = {}

                def mk(c):
                    def f():
                        if c == 0:
                            pt["kv"] = PS.tile([128, 512], f32, tag="pt",
                                               bufs=2, name="ptkv")
                        nc.tensor.matmul(
                            pt["kv"],
                            w_sb[:, c * 192 + 64:c * 192 + 192],
                            xkv_sb[:, c * NM + b * 512:c * NM + (b + 1) * 512],
                            start=(c == 0), stop=(c == 7))
                    return f

                def cp():
                    nc.vector.tensor_copy(
                        kv_sb[:, b * 512:(b + 1) * 512], pt["kv"])

                for c in range(8):
                    items.append((1, mk(c)))
                items.append((0, cp))
                for cc in range(4):
                    j = 4 * b + cc
                    items.append((1, mk_vtr(
                        j, kv_sb[64:128, j * 128:(j + 1) * 128], True)))
                return items

            # v^T chunk [64,128] -> v natural [128,64] via PE transpose,
            # then cast-copy into vn (shares the "fin" psum bank).
            def mk_vtr(j, src_ap, hi):
                # hi: source lives at partitions 64-127; identity slice must
                # share the source's base partition.
                def f():
                    tp = PS.tile([128, 65], f32, tag="fin", bufs=1,
                                 name="vtr")
                    tb = tp[:, 0:32].bitcast(mybir.dt.bfloat16)  # [128,64]
                    ident = idb_sb[64:128, 64:128] if hi else idb_sb[0:64, 0:64]
                    nc.tensor.transpose(tb, src_ap, ident)
                    nc.vector.tensor_copy(
                        vn_sb[:, j * 80:j * 80 + 64], tb)
                return f

            # ---- finalize emitter ----------------------------------------
            def fin_items(g, op):
                items = []
                st = {}

                def cp_ob():
                    st["ob"] = W.tile([65, 512], f32, tag="ob", bufs=2,
                                      name="ob")
                    nc.vector.tensor_copy(st["ob"], op[0:65, :])

                def mk_t4(t4):
                    def f():
                        tp = PS.tile([128, 65], f32, tag="fin", bufs=1,
                                     name="otr")
                        nc.tensor.transpose(
                            tp, st["ob"][:, t4 * 128:(t4 + 1) * 128],
                            idf_sb[0:65, 0:65])
                        rc = W.tile([128, 1], f32, tag="rc", bufs=2, name="rc")
                        nc.vector.reciprocal(rc, tp[:, 64:65])
                        rs = W.tile([128, 64], f32, tag="rs", bufs=2, name="rs")
                        nc.vector.tensor_scalar_mul(rs, tp[:, 0:64], rc)
                        r0 = g * QB + t4 * 128
                        nc.sync.dma_start(out=out_d[r0:r0 + 128, :], in_=rs)
                    return f

                items.append((0, cp_ob))
                for t4 in range(4):
                    items.append((1, mk_t4(t4)))
                return items

            # ---- attention -----------------------------------------------
            # Units per qb g: EMAIN[g]/2 main PAIRS (two 128-key chunks,
            # two score matmuls into one [128,1024] psum, ONE exp) then 4
            # diag singles. Diag d covers local keys [128d,128d+128); only
            # queries t >= 128d see it, so score/exp/PV shrink to W=512-128d.
            def emit_unit(g, unit, op, first, last):
                kind, t = unit
                sc = PS.tile([128, 1024], f32, tag="sc", bufs=2, name="sc")
                pb = W.tile([128, 1024], bf16, tag="pb", bufs=3, name="pb")
                qAP = qT_sb[0:64, g * QB:(g + 1) * QB]
                if kind == "pair":
                    nc.tensor.matmul(
                        sc[:, 0:512], kv_sb[0:64, t * 128:(t + 1) * 128],
                        qAP, start=True, stop=True)
                    nc.tensor.matmul(
                        sc[:, 512:1024],
                        kv_sb[0:64, (t + 1) * 128:(t + 2) * 128],
                        qAP, start=True, stop=True)
                    bias = bias_sb[:, t:t + 1] if t < 4 else 0.0
                    nc.scalar.activation(
                        pb, sc, mybir.ActivationFunctionType.Exp,
                        scale=SCALE, bias=bias)

                    def pv():
                        nc.tensor.matmul(
                            op[0:65, :], vn_sb[:, t * 80:t * 80 + 65],
                            pb[:, 0:512], start=first, stop=False,
                            skip_group_check=True)
                        nc.tensor.matmul(
                            op[0:65, :],
                            vn_sb[:, (t + 1) * 80:(t + 1) * 80 + 65],
                            pb[:, 512:1024], start=False, stop=False,
                            skip_group_check=True)
                    return pv
                d = t
                w = 512 - 128 * d
                j = 4 * g + d
                nc.tensor.matmul(
                    sc[:, 0:w], kq_sb[0:64, j * 128:(j + 1) * 128],
                    qT_sb[0:64, g * QB + 128 * d:(g + 1) * QB],
                    start=True, stop=True)
                nc.scalar.activation(
                    pb[:, 0:w], sc[:, 0:w],
                    mybir.ActivationFunctionType.Exp, scale=SCALE)
                nc.gpsimd.tensor_mul(pb[:, 0:128], pb[:, 0:128], tri_sb)

                def pv():
                    jj = MAINC + 4 * g + d
                    nc.tensor.matmul(
                        op[0:65, 128 * d:512],
                        vn_sb[:, jj * 80:jj * 80 + 65],
                        pb[:, 0:w], start=False, stop=last,
                        skip_group_check=True)
                return pv

            # ---- top-level schedule --------------------------------------
            # upfront projections
            for _, f in proj_kv_items(0):
                f()
            for _, f in proj_xq_items(0):
                f()

            fillers = []        # (weight, closure) queue, consumed in order

            def add_fillers(items):
                fillers.extend(items)

            def pump(target):
                # consume fillers until cumulative weight >= target
                while fillers and pump.done < target:
                    wgt, f = fillers.pop(0)
                    f()
                    pump.done += wgt
                while fillers and fillers[0][0] == 0:
                    _, f = fillers.pop(0)
                    f()
            pump.done = 0

            qb_fill = {
                0: lambda: proj_xq_items(1) + proj_kv_items(1)
                + proj_kv_items(2),
                1: lambda: proj_xq_items(2) + proj_kv_items(3)
                + proj_kv_items(4),
                2: lambda: proj_xq_items(3),
                3: lambda: proj_kv_items(5) + proj_kv_items(6),
            }
            # for qb3 the kv5/kv6 fillers must land before chunk-slots 20/24
            # (units 10/12): pace them over the first 9 units.
            pace_units = {0: 6, 1: 10, 2: 14, 3: 9}

            for g in range(NQB):
                op = PS.tile([128, 512], f32, tag="op", bufs=1, name="op")
                units = [("pair", t) for t in range(0, EMAIN[g], 2)] \
                    + [("diag", d) for d in range(4)]
                n = len(units)
                base = pump.done
                add_fillers(qb_fill[g]())
                wsum = sum(w for w, _ in fillers)
                pend = []   # pv closures awaiting emission (lag 2 units)
                npace = pace_units[g]
                for i, unit in enumerate(units):
                    pump(base + math.ceil(wsum * min(1.0, (i + 1) / npace)))
                    if len(pend) >= 2:
                        pend.pop(0)()
                    pend.append(emit_unit(g, unit, op, i == 0, i == n - 1))
                for pv in pend:
                    pv()
                add_fillers(fin_items(g, op))

            while fillers:
                _, f = fillers.pop(0)
                f()
    nc.compile()
    return nc


def _get_program():
    if "nc" not in _CACHE:
        _CACHE["nc"] = _build_program()
    return _CACHE["nc"]


def _host_prep(x, Wk, Wq, Wv):
    """Build the 8 per-core input maps."""
    wt = np.concatenate([Wq.T, Wk.T, Wv.T], axis=1).astype(BF16)  # [C, 192]
    xT = [np.ascontiguousarray(x[b].T).astype(BF16) for b in range(B)]
    bias_pad = np.full((128, 4), NEGBIG, dtype=np.float32)
    bias_real = np.zeros((128, 4), dtype=np.float32)
    in_maps = []
    for core in range(NCORES):
        b, p = core // 2, core % 2
        gs = [2 * i + p for i in range(NQB)]
        xq = np.concatenate(
            [xT[b][:, 512 * g:512 * g + 512] for g in gs], axis=1)
        if p == 0:
            xkv = np.concatenate(
                [np.zeros((C, 512), dtype=BF16), xT[b][:, 0:3072]], axis=1)
            bias = bias_pad
        else:
            xkv = np.ascontiguousarray(xT[b][:, 0:3584])
            bias = bias_real
        in_maps.append({
            "xq": np.ascontiguousarray(xq),
            "xkv": np.ascontiguousarray(xkv),
            "wt": np.ascontiguousarray(wt),
            "bias": bias,
        })
    return in_maps


def _gather(results):
    out = np.zeros((B, T, H), dtype=np.float32)
    for core in range(NCORES):
        b, p = core // 2, core % 2
        shard = np.asarray(results[core]["out"], dtype=np.float32)
        for i in range(NQB):
            g = 2 * i + p
            out[b, 512 * g:512 * g + 512, :] = shard[512 * i:512 * i + 512, :]
    return out


def run(x, Wk, Wq, Wv, trace=False):
    from concourse.bass_utils import run_bass_kernel_spmd

    nc = _get_program()
    in_maps = _host_prep(x, Wk, Wq, Wv)
    res = run_bass_kernel_spmd(
        nc, in_maps, list(range(NCORES)), trace=trace)
    return _gather(res.results), res


def kernel(x, Wk, Wq, Wv):
    out, _ = run(np.asarray(x, dtype=np.float32),
                 np.asarray(Wk, dtype=np.float32),
                 np.asarray(Wq, dtype=np.float32),
                 np.asarray(Wv, dtype=np.float32))
    return out
